# revision 5
# baseline (speedup 1.0000x reference)
"""Bass/Trainium2 kernel for the DST-I spectral elliptic solver (QG model).

  psi = Cm2l @ [ S (Hinv * (S (Cl2m@q) S)) S  + alpha*homsol ]

S = symmetric orthogonal 2047-point DST-I matrix; executed as 16 large
fp32r matmuls on the tensor engines, grid padded 2047->2048.

Distribution: all 4 modes on every core; spatial axis sharded 8x256.
Sharding chain  y -> kx -> ky -> x  with one AllToAll per hop, issued
per-mode so collectives overlap matmul compute. Zero-mean correction via
sum(psi) = w^T T w  (w = column sums of S) + a tiny AllReduce.
"""
import numpy as np

NZ = 4          # layers/modes
N = 2047        # logical grid
NP = 2048       # padded grid
NC = 8          # cores
SH = NP // NC   # 256 shard width
KT = NP // 128  # 16 partition tiles
CH = NP // 512  # 4 free-dim chunks

_PROG = {}


def _build_program():
    import concourse.mybir as mybir
    import concourse.tile as tile
    from concourse import bacc

    F32 = mybir.dt.float32
    F32R = mybir.dt.float32r
    MUL = mybir.AluOpType.mult
    ADD = mybir.AluOpType.add
    BYP = mybir.AluOpType.bypass
    RG = [list(range(NC))]

    nc = bacc.Bacc("TRN2", target_bir_lowering=False, debug=False, num_devices=NC)

    q_in = nc.dram_tensor("q_in", [NZ, NP, SH], F32, kind="ExternalInput")
    s_in = nc.dram_tensor("s_in", [KT, 128, NP], F32R, kind="ExternalInput")
    h_in = nc.dram_tensor("h_in", [NZ, 2, 128, NP], F32, kind="ExternalInput")
    hom_in = nc.dram_tensor("hom_in", [NZ, 2, 128, NP], F32, kind="ExternalInput")
    wrep_in = nc.dram_tensor("wrep_in", [128, NP], F32, kind="ExternalInput")
    wcol_in = nc.dram_tensor("wcol_in", [128, 2], F32, kind="ExternalInput")
    ones_in = nc.dram_tensor("ones_in", [1, 128], F32R, kind="ExternalInput")
    clm_in = nc.dram_tensor("clm_in", [128, 16], F32, kind="ExternalInput")
    cml_in = nc.dram_tensor("cml_in", [128, 16], F32, kind="ExternalInput")
    cmlT_in = nc.dram_tensor("cmlT_in", [4, 4], F32, kind="ExternalInput")
    hm_in = nc.dram_tensor("hm_in", [1, 4], F32, kind="ExternalInput")
    out_d = nc.dram_tensor("out_d", [NZ, 2, 128, NP], F32, kind="ExternalOutput")
    beta_d = nc.dram_tensor("beta_d", [4, 4], F32, kind="ExternalOutput")

    with tile.TileContext(nc) as tc:
        with (
            tc.tile_pool(name="dram", bufs=1, space="DRAM") as dram,
            tc.tile_pool(name="psum", bufs=8, space="PSUM") as psum,
            tc.tile_pool(name="const", bufs=1) as const,
            tc.tile_pool(name="tiny", bufs=1) as tiny,
        ):
            bin_ = [[dram.tile([NC, 2, 128, SH], F32R, tag=f"b{r}i{m}",
                                name=f"b{r}i{m}")
                     for m in range(NZ)] for r in range(3)]
            bout = [[dram.tile([NC, 2, 128, SH], F32R, tag=f"b{r}o{m}",
                               name=f"b{r}o{m}")
                     for m in range(NZ)] for r in range(3)]
            ar_i = dram.tile([1, 8], F32, tag="ar_i")
            ar_o = dram.tile([1, 8], F32, tag="ar_o")
            a4_d = dram.tile([1, 4], F32, tag="a4")
            b16_d = dram.tile([1, 16], F32R, tag="b16")
            e_d = dram.tile([NZ, 2, 128, NP], F32, tag="e_d")

            S_sb = const.tile([128, KT, NP], F32R, tag="S")
            for k in range(KT):
                nc.sync.dma_start(S_sb[:, k, :], s_in.ap()[k])
            wcol = const.tile([128, 2], F32, tag="wcol")
            nc.sync.dma_start(wcol[:], wcol_in.ap())
            ones = const.tile([1, 128], F32R, tag="ones")
            nc.sync.dma_start(ones[:], ones_in.ap())
            clm = const.tile([128, 16], F32, tag="clm")
            nc.sync.dma_start(clm[:], clm_in.ap())
            cml = const.tile([128, 16], F32, tag="cml")
            nc.sync.dma_start(cml[:], cml_in.ap())
            cmlT = const.tile([4, 4], F32, tag="cmlT")
            nc.sync.dma_start(cmlT[:], cmlT_in.ap())
            hm = const.tile([1, 4], F32, tag="hm")
            nc.sync.dma_start(hm[:], hm_in.ap())
            brep = const.tile([128, 16], F32, tag="brep")

            def mm_stage(lhs_fn, evict_fn):
                for n in range(CH):
                    ps = [psum.tile([128, 512], F32, tag="acc", name=f"ps{n}_{i}")
                          for i in range(2)]
                    for k in range(KT):
                        rhs = S_sb[:, k, n * 512:(n + 1) * 512]
                        for st in range(2):
                            nc.tensor.matmul(
                                ps[st][:], lhs_fn(k, st), rhs,
                                start=(k == 0), stop=(k == KT - 1))
                    evict_fn(n, ps)

            def load_resident(pool, tag, bsrc):
                t = pool.tile([128, KT, SH], F32R, tag=tag)
                for k in range(KT):
                    j, half = divmod(k, 2)
                    nc.sync.dma_start(t[:, k, :], bsrc[j, half, :, :])
                return t

            def a2a(src, dst):
                nc.gpsimd.collective_compute(
                    "AllToAll", BYP, replica_groups=RG,
                    ins=[src.opt()], outs=[dst.opt()])

            with tc.tile_pool(name="stg", bufs=6) as stg:

                def evict_plain(bdst):
                    def ev(n, ps):
                        for st in range(2):
                            t = stg.tile([128, 512], F32R, tag="stg")
                            nc.vector.tensor_copy(t[:], ps[st][:])
                            for h in range(2):
                                nc.sync.dma_start(
                                    bdst[2 * n + h, st, :, :],
                                    t[:, h * 256:(h + 1) * 256])
                    return ev

                # ---------------- premix + stage 1 ----------------
                with (
                    tc.tile_pool(name="fpool", bufs=3) as fpool,
                    tc.tile_pool(name="qpool", bufs=8) as qpool,
                ):
                    f_tiles = {}

                    def premix_pass(modes):
                        for m in modes:
                            f_tiles[m] = fpool.tile([128, KT, SH], F32R,
                                                    tag="f", name=f"f{m}")
                        for xt in range(KT):
                            qts = []
                            for l in range(NZ):
                                qt = qpool.tile([128, SH], F32, tag="q")
                                nc.sync.dma_start(
                                    qt[:],
                                    q_in.ap()[l, xt * 128:(xt + 1) * 128, :])
                                qts.append(qt)
                            for m in modes:
                                dst = f_tiles[m][:, xt, :]
                                nc.vector.tensor_scalar(
                                    dst, qts[0][:], clm[:, 4 * m:4 * m + 1],
                                    None, MUL)
                                for l in (1, 2, 3):
                                    nc.vector.scalar_tensor_tensor(
                                        dst, qts[l][:],
                                        clm[:, 4 * m + l:4 * m + l + 1],
                                        dst, MUL, ADD)

                    premix_pass([0, 1])
                    for m in range(NZ):
                        if m == 2:
                            premix_pass([2, 3])
                        f = f_tiles[m]
                        mm_stage(
                            lambda k, st, f=f: f[:, k, st * 128:(st + 1) * 128],
                            evict_plain(bin_[0][m]))
                        a2a(bin_[0][m], bout[0][m])

                # ---------------- stage 2 + Hinv + z ----------------
                with tc.tile_pool(name="stin", bufs=2) as stin:
                    psz = psum.tile([1, 4], F32, tag="acc")
                    with (
                        tc.tile_pool(name="hpool", bufs=2) as hpool,
                        tc.tile_pool(name="wpool", bufs=1) as wpool,
                        tc.tile_pool(name="zpool", bufs=10) as zpool,
                    ):
                        wrep = wpool.tile([128, NP], F32, tag="wrep")
                        nc.sync.dma_start(wrep[:], wrep_in.ap())
                        for m in range(NZ):
                            at = load_resident(stin, "stin", bout[0][m])
                            zprev = [None, None]

                            def ev2(n, ps, m=m, zprev=zprev):
                                for st in range(2):
                                    ht = hpool.tile([128, 512], F32, tag="ht")
                                    nc.sync.dma_start(
                                        ht[:],
                                        h_in.ap()[m, st, :,
                                                  n * 512:(n + 1) * 512])
                                    hs_ = hpool.tile([128, 512], F32, tag="hs")
                                    hv = hpool.tile([128, 512], F32, tag="hv")
                                    nc.vector.reciprocal_approx_accurate(
                                        out=hv[:], in_=ht[:], scratch=hs_[:])
                                    t = stg.tile([128, 512], F32R, tag="stg")
                                    nc.vector.tensor_tensor(
                                        t[:], ps[st][:], hv[:], MUL)
                                    scr = hpool.tile([128, 512], F32, tag="scr")
                                    nc.vector.tensor_tensor(
                                        scr[:], t[:].bitcast(F32),
                                        wrep[:, n * 512:(n + 1) * 512], MUL)
                                    za = zpool.tile([128, 1], F32, tag="za")
                                    nc.vector.reduce_sum(
                                        za[:], scr[:], axis=mybir.AxisListType.X)
                                    if zprev[st] is None:
                                        zprev[st] = za
                                    else:
                                        zn = zpool.tile([128, 1], F32, tag="za")
                                        nc.vector.tensor_tensor(
                                            zn[:], zprev[st][:], za[:], ADD)
                                        zprev[st] = zn
                                    for h in range(2):
                                        nc.sync.dma_start(
                                            bin_[1][m][2 * n + h, st, :, :],
                                            t[:, h * 256:(h + 1) * 256])

                            mm_stage(
                                lambda k, st, at=at:
                                    at[:, k, st * 128:(st + 1) * 128],
                                ev2)
                            for st in range(2):
                                nc.tensor.matmul(
                                    psz[0:1, m:m + 1], wcol[:, st:st + 1],
                                    zprev[st][:],
                                    start=(st == 0), stop=(st == 1))
                            a2a(bin_[1][m], bout[1][m])

                        # ---- alpha / beta ----
                        z_sb = zpool.tile([1, 4], F32, tag="zsb")
                        nc.vector.tensor_copy(z_sb[:], psz[:])
                        nc.sync.dma_start(ar_i[0:1, 0:4], z_sb[:])
                        nc.sync.dma_start(ar_i[0:1, 4:8], z_sb[:])
                        nc.gpsimd.collective_compute(
                            "AllReduce", ADD, replica_groups=RG,
                            ins=[ar_i.opt()], outs=[ar_o.opt()])
                        zsum = tiny.tile([1, 4], F32, tag="zsum")
                        nc.sync.dma_start(zsum[:], ar_o[0:1, 0:4])
                        rh = tiny.tile([1, 4], F32, tag="rh")
                        nc.vector.reciprocal(rh[:], hm[:])
                        al = tiny.tile([1, 4], F32, tag="al")
                        nc.vector.tensor_tensor(al[:], zsum[:], rh[:], MUL)
                        nc.vector.tensor_scalar(
                            al[:], al[:], -1.0 / (NP * NP), None, MUL)
                        nc.sync.dma_start(a4_d[:], al[:])
                        alc = tiny.tile([4, 1], F32, tag="alc")
                        nc.sync.dma_start(
                            alc[:], a4_d[:].rearrange("a b -> b a"))
                        bT = tiny.tile([4, 4], F32, tag="bT")
                        nc.vector.tensor_scalar(
                            bT[:], cmlT[:], alc[:, 0:1], None, MUL)
                        nc.sync.dma_start(beta_d.ap(), bT[:])
                        for r in range(4):
                            nc.sync.dma_start(
                                b16_d[0:1, 4 * r:4 * r + 4],
                                bT[r:r + 1, :].bitcast(F32R))
                        brow = tiny.tile([1, 16], F32R, tag="brow")
                        nc.sync.dma_start(brow[:], b16_d[:])
                        bps = psum.tile([128, 16], F32, tag="acc")
                        nc.tensor.matmul(bps[:], ones[:], brow[:],
                                         start=True, stop=True)
                        nc.vector.tensor_copy(brep[:], bps[:])

                    # ---------------- stage 3 + E precompute ----------------
                    with tc.tile_pool(name="epool", bufs=4) as epool:
                        for m in range(NZ):
                            tt = load_resident(stin, "stin", bout[1][m])
                            mm_stage(
                                lambda k, st, tt=tt:
                                    tt[:, k, st * 128:(st + 1) * 128],
                                evict_plain(bin_[2][m]))
                            a2a(bin_[2][m], bout[2][m])
                            # E_l = sum_m beta_lm * homsol_m ; spread the 8
                            # (xt, chunk) slots over the 4 mode iterations
                            xt = m // 2
                            for n in (2 * (m % 2), 2 * (m % 2) + 1):
                                hts = []
                                for mm in range(NZ):
                                    h_t = epool.tile([128, 512], F32, tag="eh")
                                    nc.sync.dma_start(
                                        h_t[:],
                                        hom_in.ap()[mm, xt, :,
                                                    n * 512:(n + 1) * 512])
                                    hts.append(h_t)
                                for l in range(NZ):
                                    et = epool.tile([128, 512], F32, tag="et")
                                    nc.vector.tensor_scalar(
                                        et[:], hts[0][:], brep[:, l:l + 1],
                                        None, MUL)
                                    for mm in (1, 2, 3):
                                        nc.vector.scalar_tensor_tensor(
                                            et[:], hts[mm][:],
                                            brep[:, 4 * mm + l:4 * mm + l + 1],
                                            et[:], MUL, ADD)
                                    nc.sync.dma_start(
                                        e_d[l, xt, :, n * 512:(n + 1) * 512],
                                        et[:])

            # ---------------- stage 4 + postmix ----------------
            with (
                tc.tile_pool(name="u4", bufs=3) as u4,
                tc.tile_pool(name="m3s", bufs=6) as m3s,
                tc.tile_pool(name="tmppool", bufs=6) as tmp,
                tc.tile_pool(name="erp", bufs=3) as erp,
            ):
                ur = []
                for m in range(3):
                    t_u = load_resident(u4, "u4", bout[2][m])
                    ur.append(t_u)
                for n in range(CH):
                    ps = [psum.tile([128, 512], F32, tag="acc", name=f"p4_{n}_{i}")
                          for i in range(8)]
                    for m in range(NZ):
                        for k in range(KT):
                            rhs = S_sb[:, k, n * 512:(n + 1) * 512]
                            if m == 3:
                                m3t = m3s.tile([128, SH], F32R, tag="m3")
                                j, half = divmod(k, 2)
                                nc.sync.dma_start(
                                    m3t[:], bout[2][3][j, half, :, :])
                            for xt in range(2):
                                lh = (ur[m][:, k, xt * 128:(xt + 1) * 128]
                                      if m < 3
                                      else m3t[:, xt * 128:(xt + 1) * 128])
                                nc.tensor.matmul(
                                    ps[2 * m + xt][:], lh, rhs,
                                    start=(k == 0), stop=(k == KT - 1))
                    for xt in range(2):
                        for l in range(NZ):
                            t = tmp.tile([128, 512], F32, tag="tmp")
                            nc.vector.tensor_scalar(
                                t[:], ps[xt][:], cml[:, l:l + 1], None, MUL)
                            for m in (1, 2, 3):
                                nc.vector.scalar_tensor_tensor(
                                    t[:], ps[2 * m + xt][:],
                                    cml[:, 4 * m + l:4 * m + l + 1],
                                    t[:], MUL, ADD)
                            et = erp.tile([128, 512], F32, tag="er")
                            nc.sync.dma_start(
                                et[:], e_d[l, xt, :, n * 512:(n + 1) * 512])
                            nc.vector.tensor_tensor(t[:], t[:], et[:], ADD)
                            nc.sync.dma_start(
                                out_d.ap()[l, xt, :, n * 512:(n + 1) * 512],
                                t[:])
    nc.compile()
    return nc


def _host_prep(q, Cl2m, Cm2l, H, homsol, homsol_mean):
    f32 = np.float32
    k = np.arange(1, NP, dtype=np.float64)
    S = np.sqrt(2.0 / NP) * np.sin(np.pi / NP * np.outer(k, k))
    Spad = np.zeros((NP, NP), f32)
    Spad[:N, :N] = S.astype(f32)
    w = np.zeros(NP, f32)
    w[:N] = S.sum(axis=0).astype(f32)

    s_in = np.ascontiguousarray(Spad.reshape(KT, 128, NP))
    qp = np.zeros((NZ, NP, NP), f32)
    qp[:, :N, :N] = q
    Hp = np.ones((NZ, NP, NP), f32)
    Hp[:, :N, :N] = H
    hom = np.ascontiguousarray(homsol[:, 1:NP + 1, 1:NP + 1])

    wrep = np.broadcast_to(w, (128, NP)).copy()
    ones_r = np.ones((1, 128), f32)
    clm = np.broadcast_to(Cl2m.reshape(1, 16), (128, 16)).copy().astype(f32)
    cml = np.broadcast_to(Cm2l.T.reshape(1, 16), (128, 16)).copy().astype(f32)
    cmlT = np.ascontiguousarray(Cm2l.T).astype(f32)
    hm_i = homsol_mean.reshape(1, 4).astype(f32)

    in_maps = []
    for c in range(NC):
        ys = slice(c * SH, (c + 1) * SH)
        in_maps.append({
            "q_in": np.ascontiguousarray(qp[:, :, ys]),
            "s_in": s_in,
            "h_in": np.ascontiguousarray(Hp[:, ys, :]).reshape(NZ, 2, 128, NP),
            "hom_in": np.ascontiguousarray(
                hom[:, ys, :]).reshape(NZ, 2, 128, NP),
            "wrep_in": wrep,
            "wcol_in": np.ascontiguousarray(w[ys].reshape(2, 128).T),
            "ones_in": ones_r,
            "clm_in": clm,
            "cml_in": cml,
            "cmlT_in": cmlT,
            "hm_in": hm_i,
        })
    return in_maps


def kernel(q, Cl2m, Cm2l, helmholtz_mat, homsol, homsol_mean,
           _want_results=False):
    from concourse.bass_utils import run_bass_kernel_spmd

    if "nc" not in _PROG:
        _PROG["nc"] = _build_program()
    nc = _PROG["nc"]

    in_maps = _host_prep(np.asarray(q, np.float32),
                         np.asarray(Cl2m, np.float32),
                         np.asarray(Cm2l, np.float32),
                         np.asarray(helmholtz_mat, np.float32),
                         np.asarray(homsol, np.float32),
                         np.asarray(homsol_mean, np.float32))
    res = run_bass_kernel_spmd(nc, in_maps, core_ids=list(range(NC)),
                               **_PROG.get("run_kwargs", {}))
    out = np.zeros((NZ, NP + 1, NP + 1), np.float32)
    for c in range(NC):
        core = res.results[c]["out_d"].reshape(NZ, SH, NP)
        out[:, 1 + c * SH:1 + (c + 1) * SH, 1:] = core
    beta = res.results[0]["beta_d"]  # beta[m, l] = alpha_m * Cm2l[l, m]
    hs = np.asarray(homsol, np.float32)
    out[:, 0, :] = np.einsum("ml,my->ly", beta, hs[:, 0, :])
    out[:, 1:, 0] = np.einsum("ml,mx->lx", beta, hs[:, 1:, 0])
    if _want_results:
        return out, res
    return out
